# revision 1
# baseline (speedup 1.0000x reference)
"""Trainium2 Bass kernel for MultiHead GQA attention (B=2, S=2048, D=2048,
H=16 query heads, HKV=4 kv heads, DH=128, RoPE, mask, out-proj).

Sharding: token-parallel across 8 cores. Core c handles batch c//4 and 512
query rows of it (4 blocks of 128 rows). Each core projects K/V for its whole
batch (duplicated across the 4 cores of the batch), projects Q for its rows,
runs attention + out-proj for its rows, and writes its [512, 2048] slice.
Host reassembles. No collectives.

All matmuls run in bf16 with fp32 PSUM accumulation. Host pre-transposes /
pre-tiles every operand so each DMA is a contiguous [128, X] block and each
matmul consumes operands with the contraction dim on partitions.

Attention is computed transposed: scoresT[keys, q] = khT.T @ qhT per
128-key tile, exp on ScalarE (scale folded in), probs stored bf16, then
outT[dh, q] += v_tile.T @ probsT, and row-sums via a ones-stationary matmul.
outT feeds the out-projection directly as the stationary operand.

Mask handling (host-detected, compile-time mode):
  none   - mask has no zeros: no mask work at all.
  causal - mask is exactly tril: balanced interleaved q-blocks per core +
           suffix key-ranges (only ~62% of attention tiles computed), probs
           multiplied by the exact 0/1 mask tile.
  mask   - anything else: all tiles computed, probs multiplied by 0/1 mask.
"""

import math

import numpy as np
import ml_dtypes

import concourse.bass as bass
import concourse.mybir as mybir
import concourse.tile as tile
from concourse import bacc
from concourse.bass_utils import run_bass_kernel_spmd

F32 = mybir.dt.float32
BF16 = mybir.dt.bfloat16
BF = ml_dtypes.bfloat16

B, S, D = 2, 2048, 2048
H, G = 16, 4
HKV = H // G            # 4
DH = D // H             # 128
DKV = D // G            # 512 (kv projection width)
NCORES = 8
RPC = S // 4            # 512 rows per core
NQB = RPC // 128        # 4 q-blocks of 128 rows per core
NIC = D // 128          # 16 contraction chunks
NKC = S // 128          # 16 key tiles
SCALE = 1.0 / math.sqrt(DH)

_NC_CACHE: dict = {}

# set by callers (e.g. test.py) to capture a profile; results of the last run
TRACE = False
TRACE_CORES = None          # e.g. [0] or list(range(8))
LAST_RESULTS = None


def _n_list(mode: str) -> list[int]:
    """Moving-operand width (in q columns, suffix of the 512) per key tile."""
    if mode == "causal":
        # per key-tile kc, every core keeps exactly (4 - kc//4) of its 4
        # interleaved q-blocks {r, 7-r, 8+r, 15-r} (ascending order)
        return [128 * (4 - kc // 4) for kc in range(NKC)]
    return [512] * NKC


def _build(mode: str):
    mask_mul = mode != "none"
    n_list = _n_list(mode)

    nc = bacc.Bacc("TRN2", target_bir_lowering=False, debug=False,
                   num_devices=NCORES)

    # ---- I/O (host-prepared layouts; all contiguous-DMA friendly) ----
    wq = nc.declare_dram_parameter("wq", [NIC, 128, D], BF16, isOutput=False)
    qt = nc.declare_dram_parameter("qt", [128, NIC * RPC], BF16, isOutput=False)
    # k/v: only this core's 512-token quarter (projected here, all-gathered)
    kt = nc.declare_dram_parameter("kt", [128, NIC * 512], BF16, isOutput=False)
    vt = nc.declare_dram_parameter("vt", [4, 128, NIC * 128], BF16, isOutput=False)
    wk = nc.declare_dram_parameter("wk", [HKV, 128, NIC * 128], BF16, isOutput=False)
    wv = nc.declare_dram_parameter("wv", [128, NIC * DKV], BF16, isOutput=False)
    wo = nc.declare_dram_parameter("wo", [4, 128, H * 512], BF16, isOutput=False)
    cosq = nc.declare_dram_parameter("cosq", [128, RPC], BF16, isOutput=False)
    sinq = nc.declare_dram_parameter("sinq", [128, RPC], BF16, isOutput=False)
    # cos/sin for this core's own k-token quarter
    cosk = nc.declare_dram_parameter("cosk", [128, 512], BF16, isOutput=False)
    sink = nc.declare_dram_parameter("sink", [128, 512], BF16, isOutput=False)
    pswap = nc.declare_dram_parameter("pswap", [128, 128], BF16, isOutput=False)
    if mask_mul:
        m01 = nc.declare_dram_parameter("m01", [128, NKC * RPC], BF16,
                                        isOutput=False)
    out = nc.declare_dram_parameter("out", [RPC, D], F32, isOutput=True)

    with tile.TileContext(nc) as tc:
        with (
            tc.tile_pool(name="res", bufs=1) as res,          # resident
            tc.tile_pool(name="stream2m", bufs=2) as stream2m,  # 2MB blocks
            tc.tile_pool(name="stream05", bufs=3) as stream05,  # 0.5MB blocks
            tc.tile_pool(name="small", bufs=3) as small,
            tc.tile_pool(name="probs", bufs=8) as probsp,
            tc.tile_pool(name="bcast", bufs=2) as bcastp,
            tc.tile_pool(name="dram", bufs=1, space="DRAM") as dramp,
            tc.tile_pool(name="psmm", bufs=5, space="PSUM") as psmm,
            tc.tile_pool(name="psacc", bufs=2, space="PSUM") as psacc,
            tc.tile_pool(name="pssum", bufs=1, space="PSUM") as pssum,
        ):
            # ---------------- resident tiles (DMAs staged per phase) -------
            # K-path first so the first matmul isn't stuck behind bulk loads
            coskq_t = res.tile([128, 512], BF16)
            nc.sync.dma_start(out=coskq_t, in_=cosk[:, :])
            sinkq_t = res.tile([128, 512], BF16)
            nc.sync.dma_start(out=sinkq_t, in_=sink[:, :])
            pswap_t = res.tile([128, 128], BF16)
            nc.sync.dma_start(out=pswap_t, in_=pswap[:, :])
            ones_t = res.tile([128, 1], BF16)
            nc.vector.memset(ones_t, 1.0)
            # allocated here (tag order: qts before outu_a), loaded later
            qts = res.tile([128, NIC, RPC], BF16)

            qhs = res.tile([128, H, RPC], BF16)     # rope'd q, [dh, h, rows]
            khs = res.tile([128, HKV, S], BF16)     # rope'd k, [dh, hk, keys]
            vhs = res.tile([128, 16, DKV], BF16)    # v heads, [tok%128, tokc, kv]
            # outu_a shares qts's slot: qts is dead once phase A finishes.
            # split 12/4 so phase D's early matmuls (h<12) don't dep-chain
            # behind the last normalization batch (h>=12).
            outu_a = res.tile([128, 12, RPC], BF16, tag="qts")
            outu_b = res.tile([128, 4, RPC], BF16)

            def outu(h):
                return outu_a[:, h, :] if h < 12 else outu_b[:, h - 12, :]
            # normalization batches: heads [0:8], [8:12], [12:16]
            NB = [(0, 8), (8, 12), (12, 16)]
            sums_g = [res.tile([8, RPC], F32, name=f"sums{g}", tag=f"sums{g}")
                      for g in range(len(NB))]
            rec_g = [res.tile([8, RPC], F32, name=f"rec{g}", tag=f"rec{g}")
                     for g in range(len(NB))]
            sums_dram = dramp.tile([16, RPC], F32)
            rec_dram = dramp.tile([16, RPC], F32)
            khs_own = res.tile([128, HKV, 512], BF16)
            vhs_own = res.tile([128, 4, DKV], BF16)
            kv_own = dramp.tile([2, 128, HKV, 512], BF16)
            kv_all = dramp.tile([4, 2, 128, HKV, 512], BF16)

            def rope(dst, x_bf, ps_pool, cos_ap, sin_ap, n):
                """dst = x*cos + pairswap(x)*sin  (signs baked into sin)."""
                y_ps = ps_pool.tile([128, 512], F32, tag="mm")
                # moving operand max 1024 bf16 per matmul
                assert n <= 512
                nc.tensor.matmul(y_ps[:, :n], pswap_t, x_bf, start=True,
                                 stop=True)
                t1 = small.tile([128, 512], BF16, tag="t1")
                nc.vector.tensor_mul(t1[:, :n], x_bf, cos_ap)
                t2 = small.tile([128, 512], BF16, tag="t2")
                nc.vector.tensor_mul(t2[:, :n], y_ps[:, :n], sin_ap)
                nc.vector.tensor_add(dst, t1[:, :n], t2[:, :n])

            # ------- Phase B: K/V proj for OWN 512-token quarter + RoPE -----
            # (first, so the all-gather overlaps the Q projection below)
            kmov = stream2m.tile([128, NIC, 512], BF16, tag="s2m")
            nc.sync.dma_start(out=kmov, in_=kt[:, :].rearrange(
                "p (i m) -> p i m", i=NIC))
            # cos/sin for own k-token quarter live in coskq (host-sliced)
            for hk in range(HKV):
                wk_all = stream05.tile([128, NIC, 128], BF16, tag="s05")
                nc.sync.dma_start(out=wk_all, in_=wk[hk].rearrange(
                    "p (i m) -> p i m", i=NIC))
                ps = psmm.tile([128, 512], F32, tag="mm")
                for ic in range(NIC):
                    nc.tensor.matmul(ps, wk_all[:, ic, :],
                                     kmov[:, ic, :],
                                     start=(ic == 0), stop=(ic == NIC - 1))
                xk = small.tile([128, 512], BF16, tag="xq")
                nc.scalar.copy(xk, ps)
                rope(khs_own[:, hk, :], xk, psmm, coskq_t, sinkq_t, 512)

            wvs = res.tile([128, NIC, DKV], BF16)
            nc.sync.dma_start(out=wvs, in_=wv[:, :].rearrange(
                "p (i n) -> p i n", i=NIC))
            for j in range(4):            # own 128-token blocks (V stationary)
                vmov = stream05.tile([128, NIC, 128], BF16, tag="s05")
                nc.sync.dma_start(out=vmov, in_=vt[j].rearrange(
                    "p (i m) -> p i m", i=NIC))
                ps = psmm.tile([128, 512], F32, tag="mm")
                for ic in range(NIC):
                    nc.tensor.matmul(ps, vmov[:, ic, :],
                                     wvs[:, ic, :],
                                     start=(ic == 0), stop=(ic == NIC - 1))
                nc.vector.tensor_copy(vhs_own[:, j, :], ps)

            # ---- all-gather projected K/V across the 4 cores of the batch --
            nc.sync.dma_start(out=kv_own[0], in_=khs_own)
            nc.sync.dma_start(out=kv_own[1], in_=vhs_own)
            nc.gpsimd.collective_compute(
                "AllGather", mybir.AluOpType.bypass,
                replica_groups=[[0, 1, 2, 3], [4, 5, 6, 7]],
                ins=[kv_own[:, :, :, :]], outs=[kv_all[:, :, :, :, :]])
            for r in range(4):
                nc.sync.dma_start(out=khs[:, :, r * 512:(r + 1) * 512],
                                  in_=kv_all[r, 0])
                nc.sync.dma_start(out=vhs[:, 4 * r:4 * r + 4, :],
                                  in_=kv_all[r, 1])

            # ---------------- Phase A: Q-proj + RoPE ----------------
            nc.sync.dma_start(out=qts, in_=qt[:, :].rearrange(
                "p (i m) -> p i m", i=NIC))
            cosq_t = res.tile([128, RPC], BF16)
            nc.sync.dma_start(out=cosq_t, in_=cosq[:, :])
            sinq_t = res.tile([128, RPC], BF16)
            nc.sync.dma_start(out=sinq_t, in_=sinq[:, :])
            for oc in range(H):
                wq_all = stream05.tile([128, NIC, 128], BF16, tag="s05")
                nc.sync.dma_start(out=wq_all, in_=wq[oc].rearrange(
                    "p (i m) -> p i m", i=NIC))
                ps = psmm.tile([128, 512], F32, tag="mm")
                for ic in range(NIC):
                    nc.tensor.matmul(ps, wq_all[:, ic, :],
                                     qts[:, ic, :],
                                     start=(ic == 0), stop=(ic == NIC - 1))
                xq = small.tile([128, 512], BF16, tag="xq")
                nc.scalar.copy(xq, ps)
                rope(qhs[:, oc, :], xq, psmm, cosq_t, sinq_t, RPC)

            # ---------------- Phase C: attention per head ----------------
            if mask_mul:
                m01s = res.tile([128, NKC, RPC], BF16)
                nc.sync.dma_start(out=m01s, in_=m01[:, :].rearrange(
                    "p (k m) -> p k m", k=NKC))

            def normalize_batch(g):
                """reciprocal + broadcast + in-place normalize for the heads
                of batch g (their sums are already in sums_dram)."""
                a, bnd = NB[g]
                m = bnd - a
                nc.sync.dma_start(out=sums_g[g][:m, :],
                                  in_=sums_dram[a:bnd, :])
                nc.vector.reciprocal(rec_g[g][:m, :], sums_g[g][:m, :])
                nc.sync.dma_start(out=rec_dram[a:bnd, :], in_=rec_g[g][:m, :])
                for h in range(a, bnd):
                    recb = bcastp.tile([128, RPC], F32, tag="bc")
                    nc.sync.dma_start(
                        out=recb,
                        in_=rec_dram[h:h + 1, :].to_broadcast([128, RPC]))
                    nc.vector.tensor_mul(outu(h), outu(h), recb)

            for h in range(H):
                hk = h // G
                ps_o = psacc.tile([128, 512], F32, tag="acc")
                ps_s = pssum.tile([1, 512], F32, tag="sum")
                for kc in range(NKC):
                    n = n_list[kc]
                    lo = RPC - n          # suffix columns
                    ps_sc = psmm.tile([128, 512], F32, tag="mm")
                    nc.tensor.matmul(
                        ps_sc[:, :n],
                        khs[:, hk, kc * 128:(kc + 1) * 128],
                        qhs[:, h, lo:],
                        start=True, stop=True, skip_group_check=True)
                    probs = probsp.tile([128, 512], BF16, tag="pr")
                    nc.scalar.activation(
                        probs[:, :n], ps_sc[:, :n],
                        mybir.ActivationFunctionType.Exp, scale=SCALE)
                    if mask_mul:
                        # causal: only the lowest <=2 blocks of the suffix can
                        # contain masked entries (padding + diagonal); above
                        # the diagonal every block is fully kept.
                        nm = min(256, n) if mode == "causal" else n
                        nc.vector.tensor_mul(probs[:, :nm], probs[:, :nm],
                                             m01s[:, kc, lo:lo + nm])
                    first = kc == 0
                    last = kc == NKC - 1
                    nc.tensor.matmul(ps_s[:, lo:], ones_t, probs[:, :n],
                                     start=first, stop=last,
                                     skip_group_check=True)
                    nc.tensor.matmul(
                        ps_o[:, lo:],
                        vhs[:, kc, hk * 128:(hk + 1) * 128],
                        probs[:, :n],
                        start=first, stop=last, skip_group_check=True)
                sm1 = small.tile([1, RPC], F32, tag="sm1", bufs=2)
                nc.vector.tensor_copy(sm1, ps_s)
                nc.sync.dma_start(out=sums_dram[h:h + 1, :], in_=sm1)
                nc.vector.tensor_copy(outu(h), ps_o)
                if h == 7:
                    normalize_batch(0)
                elif h == 11:
                    normalize_batch(1)
            normalize_batch(2)

            # ---------------- Phase D: out-projection ----------------
            for oc in range(4):
                wo_all = stream2m.tile([128, H, 512], BF16, tag="s2m")
                nc.sync.dma_start(out=wo_all, in_=wo[oc].rearrange(
                    "p (h m) -> p h m", h=H))
                for qc in range(NQB):
                    ps_f = psmm.tile([128, 512], F32, tag="mm")
                    for h in range(H):
                        lh = outu_a[:, h, qc * 128:(qc + 1) * 128] if h < 12 \
                            else outu_b[:, h - 12, qc * 128:(qc + 1) * 128]
                        nc.tensor.matmul(
                            ps_f, lh, wo_all[:, h, :],
                            start=(h == 0), stop=(h == H - 1))
                    fin = small.tile([128, 512], F32, tag="fin")
                    nc.vector.tensor_copy(fin, ps_f)
                    nc.sync.dma_start(
                        out=out[qc * 128:(qc + 1) * 128,
                                oc * 512:(oc + 1) * 512],
                        in_=fin)

    nc.compile()
    return nc


def _get_nc(mode: str):
    if mode not in _NC_CACHE:
        _NC_CACHE[mode] = _build(mode)
    return _NC_CACHE[mode]


def _core_rows(mode: str, r: int) -> np.ndarray:
    """Global (within-batch) q-row indices owned by quarter r, ascending."""
    if mode == "causal":
        blocks = sorted([r, 7 - r, 8 + r, 15 - r])
    else:
        blocks = [4 * r, 4 * r + 1, 4 * r + 2, 4 * r + 3]
    return np.concatenate([np.arange(b * 128, (b + 1) * 128) for b in blocks])


def kernel(q, k, v, mask, freqs, W_q, W_k, W_v, W_o):
    q = np.asarray(q, dtype=np.float32)
    k = np.asarray(k, dtype=np.float32)
    v = np.asarray(v, dtype=np.float32)
    mask = np.asarray(mask, dtype=np.float32)
    freqs = np.asarray(freqs, dtype=np.float32)
    W_q = np.asarray(W_q, dtype=np.float32)
    W_k = np.asarray(W_k, dtype=np.float32)
    W_v = np.asarray(W_v, dtype=np.float32)
    W_o = np.asarray(W_o, dtype=np.float32)

    # ---- mask mode detection ----
    nz = mask != 0
    if nz.all():
        mode = "none"
    else:
        tril = np.tril(np.ones((S, S), dtype=bool))
        mode = "causal" if all(np.array_equal(nz[b], tril) for b in range(B)) \
            else "mask"

    # ---- shared host precomputation ----
    c_full = np.cos(freqs)                      # [S, 64]
    s_full = np.sin(freqs)
    sgn = np.tile(np.array([-1.0, 1.0], np.float32), DH // 2)  # [-,+,-,+...]
    cosk_h = np.repeat(c_full, 2, axis=1).T.astype(BF)          # [128, S]
    sink_h = (np.repeat(s_full, 2, axis=1) * sgn).T.astype(BF)

    psw = np.zeros((128, 128), np.float32)
    idx = np.arange(128)
    psw[idx, idx ^ 1] = 1.0
    psw = psw.astype(BF)

    # weight layouts
    # wq[oc, p, i*128+m] = W_q[oc*128+m, i*128+p]
    wq_h = np.ascontiguousarray(
        W_q.reshape(H, 128, NIC, 128).transpose(0, 3, 2, 1)
        .reshape(H, 128, D)).astype(BF)
    # wk[hk, p, i*128+m] = W_k[hk*128+m, i*128+p]
    wk_h = np.ascontiguousarray(
        W_k.reshape(HKV, 128, NIC, 128).transpose(0, 3, 2, 1)
        .reshape(HKV, 128, D)).astype(BF)
    # wv[p, i*512+n] = W_v[n, i*128+p]
    wv_h = np.ascontiguousarray(
        W_v.reshape(DKV, NIC, 128).transpose(2, 1, 0).reshape(128, NIC * DKV)
    ).astype(BF)
    # wo[oc, p, h*512+m] = W_o[oc*512+m, h*128+p]
    wo_h = np.ascontiguousarray(
        W_o.reshape(4, 512, H, 128).transpose(0, 3, 2, 1).reshape(4, 128, -1)
    ).astype(BF)

    # k/v: each core only gets its own 512-token quarter (gathered on device)
    # kt[p, i*512+t] = k[b, tq*512+t, i*128+p] for quarter tq
    kt_b = []   # [B][4] quarters
    vt_b = []
    for b in range(B):
        kt_b.append([np.ascontiguousarray(
            k[b, tq * 512:(tq + 1) * 512].reshape(512, NIC, 128)
            .transpose(2, 1, 0).reshape(128, NIC * 512)).astype(BF)
            for tq in range(4)])
        # vt[j, p, i*128+t] = v[b, tq*512 + j*128+t, i*128+p]
        vt_b.append([np.ascontiguousarray(
            v[b, tq * 512:(tq + 1) * 512].reshape(4, 128, NIC, 128)
            .transpose(0, 3, 2, 1).reshape(4, 128, NIC * 128)).astype(BF)
            for tq in range(4)])

    in_maps = []
    rows_all = []
    for c in range(NCORES):
        b, r = divmod(c, 4)
        rows = _core_rows(mode, r)
        rows_all.append((b, rows))
        # qt[p, i*512+t] = q[b, rows[t], i*128+p]
        qsl = q[b][rows]                       # [512, D]
        qt_h = np.ascontiguousarray(
            qsl.reshape(RPC, NIC, 128).transpose(2, 1, 0).reshape(128, -1)
        ).astype(BF)
        cq = np.repeat(c_full[rows], 2, axis=1).T.astype(BF)      # [128, 512]
        sq = (np.repeat(s_full[rows], 2, axis=1) * sgn).T.astype(BF)
        im = {
            "wq": wq_h, "qt": qt_h, "kt": kt_b[b][r], "vt": vt_b[b][r],
            "wk": wk_h, "wv": wv_h, "wo": wo_h,
            "cosq": cq, "sinq": sq,
            "cosk": np.ascontiguousarray(cosk_h[:, r * 512:(r + 1) * 512]),
            "sink": np.ascontiguousarray(sink_h[:, r * 512:(r + 1) * 512]),
            "pswap": psw,
        }
        if mode != "none":
            # m01[p, kc*512+m] = (mask[b, rows[m], kc*128+p] != 0)
            msl = nz[b][rows]                  # [512, S] bool
            m01_h = np.ascontiguousarray(
                msl.T.reshape(NKC, 128, RPC).transpose(1, 0, 2)
                .reshape(128, -1)).astype(BF)
            im["m01"] = m01_h
        in_maps.append(im)

    nc = _get_nc(mode)
    kwargs = {}
    if TRACE:
        kwargs["trace"] = True
        if TRACE_CORES:
            kwargs["trace_cores"] = list(TRACE_CORES)
    results = run_bass_kernel_spmd(nc, in_maps, core_ids=list(range(NCORES)),
                                   **kwargs)
    global LAST_RESULTS
    LAST_RESULTS = results

    full = np.empty((B, S, D), np.float32)
    for c in range(NCORES):
        b, rows = rows_all[c]
        full[b, rows] = results.results[c]["out"]
    return full



# revision 8
# speedup vs baseline: 1.1082x; 1.1082x over previous
"""Trainium2 Bass kernel for MultiHead GQA attention (B=2, S=2048, D=2048,
H=16 query heads, HKV=4 kv heads, DH=128, RoPE, mask, out-proj).

Sharding: token-parallel across 8 cores. Core c handles batch c//4 and 512
query rows of it (4 blocks of 128 rows). Each core projects K/V for its own
512-token quarter, all-gathers projected K/V across the 4 cores of its batch,
runs attention + out-proj for its rows, and writes its [512, 2048] slice.
Host reassembles.

All matmuls run in bf16 with fp32 PSUM accumulation. Host pre-transposes /
pre-tiles every operand so each DMA is a contiguous [128, X] block and each
matmul consumes operands with the contraction dim on partitions.

Attention is computed transposed and software-pipelined in head-pair passes:
for each pair of query heads (same KV head), loop over 128-key tiles with
scoresT[keys, q] = khT.T @ qhT for both heads (packed into shared PSUM banks
when narrow), a single exp on ScalarE per tile-pair, 0/1 mask multiply on the
bottom 128-q stripe only (the only stripe that can be non-full under the
balanced causal interleave), softmax denominators accumulated on VectorE in
fp32 (no per-tile ones-matmuls), and outT[dh, q] += v_tile.T @ probsT lagging
one key tile behind the scores so TensorE never waits on ScalarE.

Mask handling (host-detected, compile-time mode):
  none   - mask has no zeros: no mask work at all.
  causal - mask is exactly tril: balanced interleaved q-blocks per core +
           suffix key-ranges (only ~62% of attention tiles computed), probs
           multiplied by the exact 0/1 mask on the bottom 128-q stripe.
  mask   - anything else: all tiles computed, probs multiplied by 0/1 mask.
"""

import math

import numpy as np
import ml_dtypes

import concourse.bass as bass
import concourse.mybir as mybir
import concourse.tile as tile
from concourse import bacc
from concourse.bass_utils import run_bass_kernel_spmd

F32 = mybir.dt.float32
BF16 = mybir.dt.bfloat16
BF = ml_dtypes.bfloat16

B, S, D = 2, 2048, 2048
H, G = 16, 4
HKV = H // G            # 4
DH = D // H             # 128
DKV = D // G            # 512 (kv projection width)
NCORES = 8
RPC = S // 4            # 512 rows per core
NQB = RPC // 128        # 4 q-blocks of 128 rows per core
NIC = D // 128          # 16 contraction chunks
NKC = S // 128          # 16 key tiles
SCALE = 1.0 / math.sqrt(DH)

_NC_CACHE: dict = {}

# set by callers (e.g. test.py) to capture a profile; results of the last run
TRACE = False
TRACE_CORES = None          # e.g. [0] or list(range(8))
LAST_RESULTS = None


def _n_list(mode: str) -> list[int]:
    """Moving-operand width (in q columns, suffix of the 512) per key tile."""
    if mode == "causal":
        # per key-tile kc, every core keeps exactly (4 - kc//4) of its 4
        # interleaved q-blocks {r, 7-r, 8+r, 15-r} (ascending order)
        return [128 * (4 - kc // 4) for kc in range(NKC)]
    return [512] * NKC


def _build(mode: str):
    n_list = _n_list(mode)

    nc = bacc.Bacc("TRN2", target_bir_lowering=False, debug=False,
                   num_devices=NCORES)

    # ---- I/O (host-prepared layouts; all contiguous-DMA friendly) ----
    wq = nc.declare_dram_parameter("wq", [NIC, 128, D], BF16, isOutput=False)
    qt = nc.declare_dram_parameter("qt", [128, NIC * RPC], BF16, isOutput=False)
    # k/v: only this core's 512-token quarter (projected here, all-gathered)
    kt = nc.declare_dram_parameter("kt", [128, NIC * 512], BF16, isOutput=False)
    vt = nc.declare_dram_parameter("vt", [4, 128, NIC * 128], BF16, isOutput=False)
    wk = nc.declare_dram_parameter("wk", [HKV, 128, NIC * 128], BF16, isOutput=False)
    wv = nc.declare_dram_parameter("wv", [128, NIC * DKV], BF16, isOutput=False)
    wo = nc.declare_dram_parameter("wo", [4, 128, H * 512], BF16, isOutput=False)
    cosq = nc.declare_dram_parameter("cosq", [128, RPC], BF16, isOutput=False)
    sinq = nc.declare_dram_parameter("sinq", [128, RPC], BF16, isOutput=False)
    # cos/sin for this core's own k-token quarter
    cosk = nc.declare_dram_parameter("cosk", [128, 512], BF16, isOutput=False)
    sink = nc.declare_dram_parameter("sink", [128, 512], BF16, isOutput=False)
    pswap = nc.declare_dram_parameter("pswap", [128, 128], BF16, isOutput=False)
    if mode == "causal":
        # per key tile: 0/1 mask for the bottom 128-q stripe, duplicated to
        # 256 cols so a single [128, 2, 128] strided multiply covers a pair
        m01b = nc.declare_dram_parameter("m01b", [128, NKC * 256], BF16,
                                         isOutput=False)
    elif mode == "mask":
        m01 = nc.declare_dram_parameter("m01", [128, NKC * RPC], BF16,
                                        isOutput=False)
    out = nc.declare_dram_parameter("out", [RPC, D], F32, isOutput=True)

    with tile.TileContext(nc) as tc:
        with (
            tc.tile_pool(name="res", bufs=1) as res,          # resident
            tc.tile_pool(name="stream2m", bufs=2) as stream2m,  # 2MB blocks
            tc.tile_pool(name="stream05", bufs=4) as stream05,  # 0.5MB blocks
            tc.tile_pool(name="small", bufs=3) as small,
            tc.tile_pool(name="probs", bufs=4) as probsp,
            tc.tile_pool(name="bcast", bufs=2) as bcastp,
            tc.tile_pool(name="dram", bufs=1, space="DRAM") as dramp,
            tc.tile_pool(name="psA", bufs=2, space="PSUM") as psA,
            tc.tile_pool(name="pssc", bufs=2, space="PSUM") as pssc,
            tc.tile_pool(name="psacc", bufs=2, space="PSUM") as psacc,
        ):
            # ---------------- resident tiles (DMAs staged per phase) -------
            # K-path first so the first matmul isn't stuck behind bulk loads
            coskq_t = res.tile([128, 512], BF16)
            nc.sync.dma_start(out=coskq_t, in_=cosk[:, :])
            sinkq_t = res.tile([128, 512], BF16)
            nc.sync.dma_start(out=sinkq_t, in_=sink[:, :])
            pswap_t = res.tile([128, 128], BF16)
            nc.sync.dma_start(out=pswap_t, in_=pswap[:, :])
            ones_t = res.tile([128, 1], BF16)
            nc.vector.memset(ones_t, 1.0)
            # allocated here (tag order: qts before outu_a), loaded later
            qts = res.tile([128, NIC, RPC], BF16)

            qhs = res.tile([128, H, RPC], BF16)     # rope'd q, [dh, h, rows]
            khs = res.tile([128, HKV, S], BF16)     # rope'd k, [dh, hk, keys]
            vhs = res.tile([128, 16, DKV], BF16)    # v heads, [tok%128, tokc, kv]
            # outu_a shares qts's slot: qts is dead once phase A finishes.
            outu_a = res.tile([128, 12, RPC], BF16, tag="qts")
            outu_b = res.tile([128, 4, RPC], BF16)

            def outu(h):
                return outu_a[:, h, :] if h < 12 else outu_b[:, h - 12, :]

            sums_dram = dramp.tile([16, RPC], F32)
            rec_dram = dramp.tile([16, RPC], F32)
            khs_own = res.tile([128, HKV, 512], BF16)
            vhs_own = res.tile([128, 4, DKV], BF16)
            kv_own = dramp.tile([2, 128, HKV, 512], BF16)
            kv_all = dramp.tile([4, 2, 128, HKV, 512], BF16)

            def rope(dst, x_bf, cos_ap, sin_ap, n):
                """dst = x*cos + pairswap(x)*sin  (signs baked into sin)."""
                y_ps = psacc.tile([128, 512], F32, tag="acc")
                assert n <= 512
                nc.tensor.matmul(y_ps[:, :n], pswap_t, x_bf, start=True,
                                 stop=True)
                t1 = small.tile([128, 512], BF16, tag="t1")
                nc.vector.tensor_mul(t1[:, :n], x_bf, cos_ap)
                t2 = small.tile([128, 512], BF16, tag="t2")
                nc.vector.tensor_mul(t2[:, :n], y_ps[:, :n], sin_ap)
                nc.vector.tensor_add(dst, t1[:, :n], t2[:, :n])

            # ------- Phase B: K/V proj for OWN 512-token quarter + RoPE -----
            # (first, so the all-gather overlaps the Q projection below)
            # K moving operand DMA'd in 4 chunks so the first matmul starts
            # as soon as the first 0.5MB lands.
            kmov = res.tile([128, NIC, 512], BF16)
            for part in range(4):
                nc.sync.dma_start(
                    out=kmov[:, 4 * part:4 * part + 4, :],
                    in_=kt[:, part * 4 * 512:(part + 1) * 4 * 512]
                    .rearrange("p (i m) -> p i m", i=4))
            for hk in range(HKV):
                wk_all = stream05.tile([128, NIC, 128], BF16, tag="s05w")
                nc.sync.dma_start(out=wk_all, in_=wk[hk].rearrange(
                    "p (i m) -> p i m", i=NIC))
                ps = psA.tile([128, 512], F32, tag="mm")
                for ic in range(NIC):
                    nc.tensor.matmul(ps, wk_all[:, ic, :],
                                     kmov[:, ic, :],
                                     start=(ic == 0), stop=(ic == NIC - 1))
                xk = small.tile([128, 512], BF16, tag="xq")
                nc.scalar.copy(xk, ps)
                rope(khs_own[:, hk, :], xk, coskq_t, sinkq_t, 512)

            wvs = res.tile([128, NIC, DKV], BF16)
            nc.sync.dma_start(out=wvs, in_=wv[:, :].rearrange(
                "p (i n) -> p i n", i=NIC))
            for j in range(4):            # own 128-token blocks (V stationary)
                vmov = stream05.tile([128, NIC, 128], BF16, tag="s05",
                                     bufs=2)
                nc.sync.dma_start(out=vmov, in_=vt[j].rearrange(
                    "p (i m) -> p i m", i=NIC))
                ps = psA.tile([128, 512], F32, tag="mm")
                for ic in range(NIC):
                    nc.tensor.matmul(ps, vmov[:, ic, :],
                                     wvs[:, ic, :],
                                     start=(ic == 0), stop=(ic == NIC - 1))
                nc.vector.tensor_copy(vhs_own[:, j, :], ps)

            # ---- all-gather projected K/V across the 4 cores of the batch --
            nc.sync.dma_start(out=kv_own[0], in_=khs_own)
            nc.sync.dma_start(out=kv_own[1], in_=vhs_own)
            nc.gpsimd.collective_compute(
                "AllGather", mybir.AluOpType.bypass,
                replica_groups=[[0, 1, 2, 3], [4, 5, 6, 7]],
                ins=[kv_own[:, :, :, :]], outs=[kv_all[:, :, :, :, :]])

            # ---------------- Phase A: Q-proj + RoPE ----------------
            nc.sync.dma_start(out=qts, in_=qt[:, :].rearrange(
                "p (i m) -> p i m", i=NIC))
            cosq_t = res.tile([128, RPC], BF16)
            nc.sync.dma_start(out=cosq_t, in_=cosq[:, :])
            sinq_t = res.tile([128, RPC], BF16)
            nc.sync.dma_start(out=sinq_t, in_=sinq[:, :])
            for oc in range(H):
                wq_all = stream05.tile([128, NIC, 128], BF16, tag="s05w")
                nc.sync.dma_start(out=wq_all, in_=wq[oc].rearrange(
                    "p (i m) -> p i m", i=NIC))
                ps = psA.tile([128, 512], F32, tag="mm")
                for ic in range(NIC):
                    nc.tensor.matmul(ps, wq_all[:, ic, :],
                                     qts[:, ic, :],
                                     start=(ic == 0), stop=(ic == NIC - 1))
                xq = small.tile([128, 512], BF16, tag="xq")
                nc.scalar.copy(xq, ps)
                rope(qhs[:, oc, :], xq, cosq_t, sinq_t, RPC)

            # ---- unpack all-gathered K/V (emitted after phase A so these
            # DMAs never head-of-line-block the Q-proj weight stream) ----
            for r in range(4):
                nc.sync.dma_start(out=khs[:, :, r * 512:(r + 1) * 512],
                                  in_=kv_all[r, 0])
                nc.sync.dma_start(out=vhs[:, 4 * r:4 * r + 4, :],
                                  in_=kv_all[r, 1])

            # ---------------- Phase C: attention, head-pair passes ---------
            if mode == "causal":
                # reuses kmov's slot (kmov is dead after the K projection)
                m01b_s = res.tile([128, NKC, 256], BF16, tag="kmov")
                nc.sync.dma_start(out=m01b_s, in_=m01b[:, :].rearrange(
                    "p (k m) -> p k m", k=NKC))
            elif mode == "mask":
                m01s = res.tile([128, NKC, RPC], BF16, tag="kmov")
                nc.sync.dma_start(out=m01s, in_=m01[:, :].rearrange(
                    "p (k m) -> p k m", k=NKC))

            def norm_pass(p):
                """reciprocal + broadcast + in-place normalize for the two
                heads of pass p (their sums are already in sums_dram)."""
                h0, h1 = 2 * p, 2 * p + 1
                rec = small.tile([128, 8], F32, tag="rec", bufs=2)
                nc.sync.dma_start(
                    out=rec,
                    in_=sums_dram[h0:h1 + 1, :].rearrange(
                        "a (p c) -> (a p) c", p=64))
                nc.vector.reciprocal(rec, rec)
                nc.sync.dma_start(
                    out=rec_dram[h0:h1 + 1, :].rearrange(
                        "a (p c) -> (a p) c", p=64), in_=rec)
                for h in (h0, h1):
                    recb = bcastp.tile([128, RPC], F32, tag="bc")
                    nc.sync.dma_start(
                        out=recb,
                        in_=rec_dram[h:h + 1, :].to_broadcast([128, RPC]))
                    nc.vector.tensor_mul(outu(h), outu(h), recb)

            # prefetch first two out-proj weight blocks during attention
            wo_tiles = {}

            for P in range(H // 2):
                h0, h1 = 2 * P, 2 * P + 1
                hk = h0 // G
                ps_o = [psacc.tile([128, 512], F32, tag="acc",
                                   name=f"ps_o{g}_{P}") for g in range(2)]
                # denominator accumulator; alternates between the slots of
                # khs_own/vhs_own (both dead after the all-gather send)
                acc = res.tile([128, 2, RPC], F32, name=f"acc{P}",
                               tag=("khs_own" if P % 2 == 0 else "vhs_own"))
                prev = None          # (kc, lo, n, pv) awaiting attnV
                for kc in range(NKC):
                    n = n_list[kc]
                    lo = RPC - n
                    kh_t = khs[:, hk, kc * 128:(kc + 1) * 128]
                    sc = pssc.tile([128, 2, 512], F32, tag="sc")
                    packed = n <= 256
                    if not packed:
                        nc.tensor.matmul(sc[:, 0, :n], kh_t, qhs[:, h0, lo:],
                                         start=True, stop=True,
                                         skip_group_check=True)
                        nc.tensor.matmul(sc[:, 1, :n], kh_t, qhs[:, h1, lo:],
                                         start=True, stop=True,
                                         skip_group_check=True)
                    else:
                        nc.tensor.matmul(sc[:, 0, 0:n], kh_t, qhs[:, h0, lo:],
                                         start=True, stop=True,
                                         skip_group_check=True)
                        nc.tensor.matmul(sc[:, 0, n:2 * n], kh_t,
                                         qhs[:, h1, lo:],
                                         start=True, stop=True,
                                         skip_group_check=True)
                    probs = probsp.tile([128, 2, 512], BF16, tag="pr")
                    if not packed:
                        nc.scalar.activation(
                            probs[:, :, :n], sc[:, :, :n],
                            mybir.ActivationFunctionType.Exp, scale=SCALE)
                        pv = probs[:, :, :n]
                    else:
                        nc.scalar.activation(
                            probs[:, 0, :2 * n], sc[:, 0, :2 * n],
                            mybir.ActivationFunctionType.Exp, scale=SCALE)
                        pv = probs[:, 0, :2 * n].rearrange(
                            "p (g m) -> p g m", g=2)
                    if mode == "causal":
                        nc.vector.tensor_mul(
                            pv[:, :, 0:128], pv[:, :, 0:128],
                            m01b_s[:, kc, :].rearrange(
                                "p (g m) -> p g m", g=2))
                    elif mode == "mask":
                        for g in range(2):
                            nc.vector.tensor_mul(pv[:, g, :], pv[:, g, :],
                                                 m01s[:, kc, lo:])
                    if kc == 0:
                        nc.vector.tensor_copy(acc, pv)
                    else:
                        nc.vector.tensor_add(acc[:, :, lo:], acc[:, :, lo:],
                                             pv)
                    if prev is not None:
                        pkc, plo, pn, ppv = prev
                        for g in range(2):
                            nc.tensor.matmul(
                                ps_o[g][:, plo:],
                                vhs[:, pkc, hk * 128:(hk + 1) * 128],
                                ppv[:, g, :],
                                start=(pkc == 0), stop=(pkc == NKC - 1),
                                skip_group_check=True)
                    prev = (kc, lo, n, pv)
                    if kc == 6 and P >= 2:
                        norm_pass(P - 2)    # overlapped DRAM roundtrip
                pkc, plo, pn, ppv = prev
                for g in range(2):
                    nc.tensor.matmul(
                        ps_o[g][:, plo:],
                        vhs[:, pkc, hk * 128:(hk + 1) * 128],
                        ppv[:, g, :],
                        start=(pkc == 0), stop=(pkc == NKC - 1),
                        skip_group_check=True)
                # softmax denominators: cast f32 acc once, one small matmul
                accb = probsp.tile([128, 2, 512], BF16, tag="ab", bufs=2)
                nc.vector.tensor_copy(accb, acc)
                for g, h in ((0, h0), (1, h1)):
                    ps_t = psA.tile([128, 512], F32, tag="mm")
                    nc.tensor.matmul(ps_t[0:1, :], ones_t, accb[:, g, :],
                                     start=True, stop=True,
                                     skip_group_check=True)
                    sm1 = small.tile([1, RPC], F32, tag="sm1", bufs=2)
                    nc.vector.tensor_copy(sm1, ps_t[0:1, :])
                    nc.sync.dma_start(out=sums_dram[h:h + 1, :], in_=sm1)
                    nc.vector.tensor_copy(outu(h), ps_o[g])
                if P == 5:
                    # prefetch the first out-proj weight block
                    wo_t = stream2m.tile([128, H, 512], BF16, tag="s2m")
                    nc.sync.dma_start(out=wo_t, in_=wo[0].rearrange(
                        "p (h m) -> p h m", h=H))
                    wo_tiles[0] = wo_t
            norm_pass(6)
            norm_pass(7)

            # ---------------- Phase D: out-projection ----------------
            for oc in range(4):
                if oc in wo_tiles:
                    wo_all = wo_tiles[oc]
                else:
                    wo_all = stream2m.tile([128, H, 512], BF16, tag="s2m")
                    nc.sync.dma_start(out=wo_all, in_=wo[oc].rearrange(
                        "p (h m) -> p h m", h=H))
                for qc in range(NQB):
                    ps_f = psA.tile([128, 512], F32, tag="mm")
                    for h in range(H):
                        lh = outu_a[:, h, qc * 128:(qc + 1) * 128] if h < 12 \
                            else outu_b[:, h - 12, qc * 128:(qc + 1) * 128]
                        nc.tensor.matmul(
                            ps_f, lh, wo_all[:, h, :],
                            start=(h == 0), stop=(h == H - 1))
                    fin = small.tile([128, 512], F32, tag="fin", bufs=2)
                    nc.vector.tensor_copy(fin, ps_f)
                    nc.sync.dma_start(
                        out=out[qc * 128:(qc + 1) * 128,
                                oc * 512:(oc + 1) * 512],
                        in_=fin)

    nc.compile()
    return nc


def _get_nc(mode: str):
    if mode not in _NC_CACHE:
        _NC_CACHE[mode] = _build(mode)
    return _NC_CACHE[mode]


def _core_rows(mode: str, r: int) -> np.ndarray:
    """Global (within-batch) q-row indices owned by quarter r, ascending."""
    if mode == "causal":
        blocks = sorted([r, 7 - r, 8 + r, 15 - r])
    else:
        blocks = [4 * r, 4 * r + 1, 4 * r + 2, 4 * r + 3]
    return np.concatenate([np.arange(b * 128, (b + 1) * 128) for b in blocks])


def kernel(q, k, v, mask, freqs, W_q, W_k, W_v, W_o):
    q = np.asarray(q, dtype=np.float32)
    k = np.asarray(k, dtype=np.float32)
    v = np.asarray(v, dtype=np.float32)
    mask = np.asarray(mask, dtype=np.float32)
    freqs = np.asarray(freqs, dtype=np.float32)
    W_q = np.asarray(W_q, dtype=np.float32)
    W_k = np.asarray(W_k, dtype=np.float32)
    W_v = np.asarray(W_v, dtype=np.float32)
    W_o = np.asarray(W_o, dtype=np.float32)

    # ---- mask mode detection ----
    nz = mask != 0
    if nz.all():
        mode = "none"
    else:
        tril = np.tril(np.ones((S, S), dtype=bool))
        mode = "causal" if all(np.array_equal(nz[b], tril) for b in range(B)) \
            else "mask"
    n_list = _n_list(mode)

    # ---- shared host precomputation ----
    c_full = np.cos(freqs)                      # [S, 64]
    s_full = np.sin(freqs)
    sgn = np.tile(np.array([-1.0, 1.0], np.float32), DH // 2)  # [-,+,-,+...]
    cosk_h = np.repeat(c_full, 2, axis=1).T.astype(BF)          # [128, S]
    sink_h = (np.repeat(s_full, 2, axis=1) * sgn).T.astype(BF)

    psw = np.zeros((128, 128), np.float32)
    idx = np.arange(128)
    psw[idx, idx ^ 1] = 1.0
    psw = psw.astype(BF)

    # weight layouts
    # wq[oc, p, i*128+m] = W_q[oc*128+m, i*128+p]
    wq_h = np.ascontiguousarray(
        W_q.reshape(H, 128, NIC, 128).transpose(0, 3, 2, 1)
        .reshape(H, 128, D)).astype(BF)
    # wk[hk, p, i*128+m] = W_k[hk*128+m, i*128+p]
    wk_h = np.ascontiguousarray(
        W_k.reshape(HKV, 128, NIC, 128).transpose(0, 3, 2, 1)
        .reshape(HKV, 128, D)).astype(BF)
    # wv[p, i*512+n] = W_v[n, i*128+p]
    wv_h = np.ascontiguousarray(
        W_v.reshape(DKV, NIC, 128).transpose(2, 1, 0).reshape(128, NIC * DKV)
    ).astype(BF)
    # wo[oc, p, h*512+m] = W_o[oc*512+m, h*128+p]
    wo_h = np.ascontiguousarray(
        W_o.reshape(4, 512, H, 128).transpose(0, 3, 2, 1).reshape(4, 128, -1)
    ).astype(BF)

    # k/v: each core only gets its own 512-token quarter (gathered on device)
    # kt[p, i*512+t] = k[b, tq*512+t, i*128+p] for quarter tq
    kt_b = []   # [B][4] quarters
    vt_b = []
    for b in range(B):
        kt_b.append([np.ascontiguousarray(
            k[b, tq * 512:(tq + 1) * 512].reshape(512, NIC, 128)
            .transpose(2, 1, 0).reshape(128, NIC * 512)).astype(BF)
            for tq in range(4)])
        # vt[j, p, i*128+t] = v[b, tq*512 + j*128+t, i*128+p]
        vt_b.append([np.ascontiguousarray(
            v[b, tq * 512:(tq + 1) * 512].reshape(4, 128, NIC, 128)
            .transpose(0, 3, 2, 1).reshape(4, 128, NIC * 128)).astype(BF)
            for tq in range(4)])

    in_maps = []
    rows_all = []
    for c in range(NCORES):
        b, r = divmod(c, 4)
        rows = _core_rows(mode, r)
        rows_all.append((b, rows))
        # qt[p, i*512+t] = q[b, rows[t], i*128+p]
        qsl = q[b][rows]                       # [512, D]
        qt_h = np.ascontiguousarray(
            qsl.reshape(RPC, NIC, 128).transpose(2, 1, 0).reshape(128, -1)
        ).astype(BF)
        cq = np.repeat(c_full[rows], 2, axis=1).T.astype(BF)      # [128, 512]
        sq = (np.repeat(s_full[rows], 2, axis=1) * sgn).T.astype(BF)
        im = {
            "wq": wq_h, "qt": qt_h, "kt": kt_b[b][r], "vt": vt_b[b][r],
            "wk": wk_h, "wv": wv_h, "wo": wo_h,
            "cosq": cq, "sinq": sq,
            "cosk": np.ascontiguousarray(cosk_h[:, r * 512:(r + 1) * 512]),
            "sink": np.ascontiguousarray(sink_h[:, r * 512:(r + 1) * 512]),
            "pswap": psw,
        }
        if mode == "causal":
            # m01b[p, kc*256 + g*128 + j] = mask[b, rows[lo_kc + j], kc*128+p]
            # (bottom 128-q stripe of the kept suffix, duplicated for pairs)
            msl = nz[b][rows]                  # [512, S] bool
            m01b_h = np.empty((128, NKC * 256), np.float32)
            for kc in range(NKC):
                lo = RPC - n_list[kc]
                pat = msl[lo:lo + 128, kc * 128:(kc + 1) * 128].T  # [128k,128q]
                m01b_h[:, kc * 256:kc * 256 + 128] = pat
                m01b_h[:, kc * 256 + 128:(kc + 1) * 256] = pat
            im["m01b"] = m01b_h.astype(BF)
        elif mode == "mask":
            # m01[p, kc*512+m] = (mask[b, rows[m], kc*128+p] != 0)
            msl = nz[b][rows]                  # [512, S] bool
            m01_h = np.ascontiguousarray(
                msl.T.reshape(NKC, 128, RPC).transpose(1, 0, 2)
                .reshape(128, -1)).astype(BF)
            im["m01"] = m01_h
        in_maps.append(im)

    nc = _get_nc(mode)
    kwargs = {}
    if TRACE:
        kwargs["trace"] = True
        if TRACE_CORES:
            kwargs["trace_cores"] = list(TRACE_CORES)
    results = run_bass_kernel_spmd(nc, in_maps, core_ids=list(range(NCORES)),
                                   **kwargs)
    global LAST_RESULTS
    LAST_RESULTS = results

    full = np.empty((B, S, D), np.float32)
    for c in range(NCORES):
        b, rows = rows_all[c]
        full[b, rows] = results.results[c]["out"]
    return full


# revision 14
# speedup vs baseline: 1.2722x; 1.1480x over previous
"""Trainium2 Bass kernel for MultiHead GQA attention (B=2, S=2048, D=2048,
H=16 query heads, HKV=4 kv heads, DH=128, RoPE, mask, out-proj).

Sharding: token-parallel across 8 cores. Core c handles batch c//4 and 512
query rows of it (4 blocks of 128 rows). Each core projects K/V for its own
512-token quarter, all-gathers projected K/V across the 4 cores of its batch,
runs attention + out-proj for its rows, and writes its [512, 2048] slice.
Host reassembles.

All matmuls run in bf16 with fp32 PSUM accumulation. Host pre-transposes /
pre-tiles every operand so each DMA is a contiguous [128, X] block and each
matmul consumes operands with the contraction dim on partitions.

Attention is computed transposed and software-pipelined in head-pair passes:
for each pair of query heads (same KV head), loop over 128-key tiles with
scoresT[keys, q] = khT.T @ qhT for both heads (packed into shared PSUM banks
when narrow), a single exp on ScalarE per tile-pair, 0/1 mask multiply on the
bottom 128-q stripe only (the only stripe that can be non-full under the
balanced causal interleave), softmax denominators accumulated on VectorE in
fp32 (no per-tile ones-matmuls), and outT[dh, q] += v_tile.T @ probsT lagging
one key tile behind the scores so TensorE never waits on ScalarE.

Mask handling (host-detected, compile-time mode):
  none   - mask has no zeros: no mask work at all.
  causal - mask is exactly tril: balanced interleaved q-blocks per core +
           suffix key-ranges (only ~62% of attention tiles computed), probs
           multiplied by the exact 0/1 mask on the bottom 128-q stripe.
  mask   - anything else: all tiles computed, probs multiplied by 0/1 mask.
"""

import math

import numpy as np
import ml_dtypes

import concourse.bass as bass
import concourse.mybir as mybir
import concourse.tile as tile
from concourse import bacc
from concourse.bass_utils import run_bass_kernel_spmd

F32 = mybir.dt.float32
BF16 = mybir.dt.bfloat16
BF = ml_dtypes.bfloat16

B, S, D = 2, 2048, 2048
H, G = 16, 4
HKV = H // G            # 4
DH = D // H             # 128
DKV = D // G            # 512 (kv projection width)
NCORES = 8
RPC = S // 4            # 512 rows per core
NQB = RPC // 128        # 4 q-blocks of 128 rows per core
NIC = D // 128          # 16 contraction chunks
NKC = S // 128          # 16 key tiles
SCALE = 1.0 / math.sqrt(DH)

_NC_CACHE: dict = {}

# set by callers (e.g. test.py) to capture a profile; results of the last run
TRACE = False
TRACE_CORES = None          # e.g. [0] or list(range(8))
LAST_RESULTS = None


def _n_list(mode: str) -> list[int]:
    """Moving-operand width (in q columns, suffix of the 512) per key tile."""
    if mode == "causal":
        # per key-tile kc, every core keeps exactly (4 - kc//4) of its 4
        # interleaved q-blocks {r, 7-r, 8+r, 15-r} (ascending order)
        return [128 * (4 - kc // 4) for kc in range(NKC)]
    return [512] * NKC


def _build(mode: str):
    n_list = _n_list(mode)

    nc = bacc.Bacc("TRN2", target_bir_lowering=False, debug=False,
                   num_devices=NCORES)

    # ---- I/O (host-prepared layouts; all contiguous-DMA friendly) ----
    wq = nc.declare_dram_parameter("wq", [NIC, 128, D], BF16, isOutput=False)
    qt = nc.declare_dram_parameter("qt", [128, NIC * RPC], BF16, isOutput=False)
    # k/v: only this core's 512-token quarter (projected here, all-gathered)
    kt = nc.declare_dram_parameter("kt", [128, NIC * 512], BF16, isOutput=False)
    vt = nc.declare_dram_parameter("vt", [4, 128, NIC * 128], BF16, isOutput=False)
    wk = nc.declare_dram_parameter("wk", [HKV, 128, NIC * 128], BF16, isOutput=False)
    wv = nc.declare_dram_parameter("wv", [128, NIC * DKV], BF16, isOutput=False)
    wo = nc.declare_dram_parameter("wo", [4, 128, H * 512], BF16, isOutput=False)
    cosq = nc.declare_dram_parameter("cosq", [128, RPC], BF16, isOutput=False)
    sinq = nc.declare_dram_parameter("sinq", [128, RPC], BF16, isOutput=False)
    # cos/sin for this core's own k-token quarter
    cosk = nc.declare_dram_parameter("cosk", [128, 512], BF16, isOutput=False)
    sink = nc.declare_dram_parameter("sink", [128, 512], BF16, isOutput=False)
    pswap = nc.declare_dram_parameter("pswap", [128, 128], BF16, isOutput=False)
    if mode == "causal":
        # per key tile: 0/1 mask for the bottom 128-q stripe, duplicated to
        # 256 cols so a single [128, 2, 128] strided multiply covers a pair
        m01b = nc.declare_dram_parameter("m01b", [128, NKC * 256], BF16,
                                         isOutput=False)
    elif mode == "mask":
        m01 = nc.declare_dram_parameter("m01", [128, NKC * RPC], BF16,
                                        isOutput=False)
    out = nc.declare_dram_parameter("out", [RPC, D], F32, isOutput=True)

    with tile.TileContext(nc) as tc:
        with (
            tc.tile_pool(name="res", bufs=1) as res,          # resident
            tc.tile_pool(name="stream2m", bufs=2) as stream2m,  # 2MB blocks
            tc.tile_pool(name="stream05", bufs=4) as stream05,  # 0.5MB blocks
            tc.tile_pool(name="small", bufs=3) as small,
            tc.tile_pool(name="probs", bufs=4) as probsp,
            tc.tile_pool(name="bcast", bufs=2) as bcastp,
            tc.tile_pool(name="dram", bufs=1, space="DRAM") as dramp,
            tc.tile_pool(name="psA", bufs=2, space="PSUM") as psA,
            tc.tile_pool(name="pssc", bufs=2, space="PSUM") as pssc,
            tc.tile_pool(name="psacc", bufs=2, space="PSUM") as psacc,
        ):
            # ---------------- resident tiles (DMAs staged per phase) -------
            # K-path first so the first matmul isn't stuck behind bulk loads
            coskq_t = res.tile([128, 512], BF16)
            nc.sync.dma_start(out=coskq_t, in_=cosk[:, :])
            sinkq_t = res.tile([128, 512], BF16)
            nc.sync.dma_start(out=sinkq_t, in_=sink[:, :])
            pswap_t = res.tile([128, 128], BF16)
            nc.sync.dma_start(out=pswap_t, in_=pswap[:, :])
            ones_t = res.tile([128, 1], BF16)
            nc.vector.memset(ones_t, 1.0)
            # allocated here (tag order: qts before outu_a), loaded later
            qts = res.tile([128, NIC, RPC], BF16)

            qhs = res.tile([128, H, RPC], BF16)     # rope'd q, [dh, h, rows]
            khs = res.tile([128, HKV, S], BF16)     # rope'd k, [dh, hk, keys]
            vhs = res.tile([128, 16, DKV], BF16)    # v heads, [tok%128, tokc, kv]
            # outu_a shares qts's slot: qts is dead once phase A finishes.
            outu_a = res.tile([128, 12, RPC], BF16, tag="qts")
            outu_b = res.tile([128, 4, RPC], BF16)

            def outu(h):
                return outu_a[:, h, :] if h < 12 else outu_b[:, h - 12, :]

            sums_dram = dramp.tile([16, RPC], F32)
            rec_dram = dramp.tile([16, RPC], F32)
            khs_own = res.tile([128, HKV, 512], BF16)
            vhs_own = res.tile([128, 4, DKV], BF16)
            kv_own = dramp.tile([2, 128, HKV, 512], BF16)
            kv_all = dramp.tile([4, 2, 128, HKV, 512], BF16)

            def rope(dst, x_bf, cos_ap, sin_ap, n):
                """dst = x*cos + pairswap(x)*sin  (signs baked into sin)."""
                y_ps = psacc.tile([128, 512], F32, tag="acc")
                assert n <= 512
                nc.tensor.matmul(y_ps[:, :n], pswap_t, x_bf, start=True,
                                 stop=True)
                t1 = small.tile([128, 512], BF16, tag="t1")
                nc.vector.tensor_mul(t1[:, :n], x_bf, cos_ap)
                t2 = small.tile([128, 512], BF16, tag="t2")
                nc.vector.tensor_mul(t2[:, :n], y_ps[:, :n], sin_ap)
                nc.vector.tensor_add(dst, t1[:, :n], t2[:, :n])

            # ------- Phase B: K/V proj for OWN 512-token quarter + RoPE -----
            # (first, so the all-gather overlaps the Q projection below)
            # K moving operand DMA'd in 4 chunks so the first matmul starts
            # as soon as the first chunk and weight block land.
            kmov = res.tile([128, NIC, 512], BF16)
            nc.sync.dma_start(
                out=kmov[:, 0:4, :],
                in_=kt[:, 0:4 * 512].rearrange("p (i m) -> p i m", i=4))
            wk0 = stream05.tile([128, NIC, 128], BF16, tag="s05w")
            nc.sync.dma_start(out=wk0, in_=wk[0].rearrange(
                "p (i m) -> p i m", i=NIC))
            for part in range(1, 4):
                nc.sync.dma_start(
                    out=kmov[:, 4 * part:4 * part + 4, :],
                    in_=kt[:, part * 4 * 512:(part + 1) * 4 * 512]
                    .rearrange("p (i m) -> p i m", i=4))
            for hk in range(HKV):
                if hk == 0:
                    wk_all = wk0
                else:
                    wk_all = stream05.tile([128, NIC, 128], BF16, tag="s05w")
                    nc.sync.dma_start(out=wk_all, in_=wk[hk].rearrange(
                        "p (i m) -> p i m", i=NIC))
                ps = psA.tile([128, 512], F32, tag="mm")
                for ic in range(NIC):
                    nc.tensor.matmul(ps, wk_all[:, ic, :],
                                     kmov[:, ic, :],
                                     start=(ic == 0), stop=(ic == NIC - 1))
                xk = small.tile([128, 512], BF16, tag="xq")
                nc.scalar.copy(xk, ps)
                rope(khs_own[:, hk, :], xk, coskq_t, sinkq_t, 512)

            wvs = res.tile([128, NIC, DKV], BF16)
            nc.sync.dma_start(out=wvs, in_=wv[:, :].rearrange(
                "p (i n) -> p i n", i=NIC))
            for j in range(4):            # own 128-token blocks (V stationary)
                vmov = stream05.tile([128, NIC, 128], BF16, tag="s05",
                                     bufs=2)
                nc.sync.dma_start(out=vmov, in_=vt[j].rearrange(
                    "p (i m) -> p i m", i=NIC))
                ps = psA.tile([128, 512], F32, tag="mm")
                for ic in range(NIC):
                    nc.tensor.matmul(ps, vmov[:, ic, :],
                                     wvs[:, ic, :],
                                     start=(ic == 0), stop=(ic == NIC - 1))
                nc.vector.tensor_copy(vhs_own[:, j, :], ps)

            # stage projected K/V for the all-gather (the collective itself is
            # emitted after phase A: its mesh events occupy the Sync engine
            # for the whole transfer, which would otherwise stall the kicks
            # of phase A's weight-stream DMAs)
            nc.sync.dma_start(out=kv_own[0], in_=khs_own)
            nc.sync.dma_start(out=kv_own[1], in_=vhs_own)

            # ---------------- Phase A: Q-proj + RoPE ----------------
            nc.sync.dma_start(out=qts, in_=qt[:, :].rearrange(
                "p (i m) -> p i m", i=NIC))
            cosq_t = res.tile([128, RPC], BF16)
            nc.sync.dma_start(out=cosq_t, in_=cosq[:, :])
            sinq_t = res.tile([128, RPC], BF16)
            nc.sync.dma_start(out=sinq_t, in_=sinq[:, :])
            for oc in range(H):
                wq_all = stream05.tile([128, NIC, 128], BF16, tag="s05w")
                nc.sync.dma_start(out=wq_all, in_=wq[oc].rearrange(
                    "p (i m) -> p i m", i=NIC))
                ps = psA.tile([128, 512], F32, tag="mm")
                for ic in range(NIC):
                    nc.tensor.matmul(ps, wq_all[:, ic, :],
                                     qts[:, ic, :],
                                     start=(ic == 0), stop=(ic == NIC - 1))
                xq = small.tile([128, 512], BF16, tag="xq")
                nc.scalar.copy(xq, ps)
                rope(qhs[:, oc, :], xq, cosq_t, sinq_t, RPC)

            # ---- all-gather projected K/V across the 4 cores of the batch --
            nc.gpsimd.collective_compute(
                "AllGather", mybir.AluOpType.bypass,
                replica_groups=[[0, 1, 2, 3], [4, 5, 6, 7]],
                ins=[kv_own[:, :, :, :]], outs=[kv_all[:, :, :, :, :]])
            for r in range(4):
                nc.sync.dma_start(out=khs[:, :, r * 512:(r + 1) * 512],
                                  in_=kv_all[r, 0])
                nc.sync.dma_start(out=vhs[:, 4 * r:4 * r + 4, :],
                                  in_=kv_all[r, 1])

            # ---------------- Phase C: attention, head-pair passes ---------
            if mode == "causal":
                # reuses kmov's slot (kmov is dead after the K projection)
                m01b_s = res.tile([128, NKC, 256], BF16, tag="kmov")
                nc.sync.dma_start(out=m01b_s, in_=m01b[:, :].rearrange(
                    "p (k m) -> p k m", k=NKC))
            elif mode == "mask":
                m01s = res.tile([128, NKC, RPC], BF16, tag="kmov")
                nc.sync.dma_start(out=m01s, in_=m01[:, :].rearrange(
                    "p (k m) -> p k m", k=NKC))

            def norm_pass(p):
                """reciprocal + broadcast + in-place normalize for the two
                heads of pass p (their sums are already in sums_dram)."""
                h0, h1 = 2 * p, 2 * p + 1
                rec = small.tile([128, 8], F32, tag="rec", bufs=2)
                nc.sync.dma_start(
                    out=rec,
                    in_=sums_dram[h0:h1 + 1, :].rearrange(
                        "a (p c) -> (a p) c", p=64))
                nc.vector.reciprocal(rec, rec)
                nc.sync.dma_start(
                    out=rec_dram[h0:h1 + 1, :].rearrange(
                        "a (p c) -> (a p) c", p=64), in_=rec)
                for h in (h0, h1):
                    recb = bcastp.tile([128, RPC], F32, tag="bc")
                    nc.sync.dma_start(
                        out=recb,
                        in_=rec_dram[h:h + 1, :].to_broadcast([128, RPC]))
                    nc.vector.tensor_mul(outu(h), outu(h), recb)

            # prefetch first two out-proj weight blocks during attention
            wo_tiles = {}

            def rowsums(P, acc):
                """denominator row-sums for pass P (emitted one pass later so
                the tiny matmuls never stall TensorE at a pass boundary)."""
                for g, h in ((0, 2 * P), (1, 2 * P + 1)):
                    ps_t = psA.tile([128, 512], F32, tag="mm",
                                    name=f"ps_t{g}_{P}")
                    nc.tensor.matmul(ps_t[0:1, :], ones_t, acc[:, g, :],
                                     start=True, stop=True,
                                     skip_group_check=True)
                    sm1 = small.tile([1, RPC], F32, tag="sm1", bufs=2)
                    nc.vector.tensor_copy(sm1, ps_t[0:1, :])
                    nc.sync.dma_start(out=sums_dram[h:h + 1, :], in_=sm1)

            prev_acc = None
            for P in range(H // 2):
                h0, h1 = 2 * P, 2 * P + 1
                hk = h0 // G
                ps_o = [psacc.tile([128, 512], F32, tag="acc",
                                   name=f"ps_o{g}_{P}") for g in range(2)]
                # denominator accumulator (bf16: summands are positive and
                # only ~16 per column, rounding stays within tolerance);
                # alternates between the slots of khs_own/vhs_own (both dead
                # after the all-gather staging DMAs)
                acc = res.tile([128, 2, RPC], BF16, name=f"acc{P}",
                               tag=("khs_own" if P % 2 == 0 else "vhs_own"))
                prev = None          # (kc, lo, n, pv) awaiting attnV
                for kc in range(NKC):
                    n = n_list[kc]
                    lo = RPC - n
                    kh_t = khs[:, hk, kc * 128:(kc + 1) * 128]
                    sc = pssc.tile([128, 2, 512], F32, tag="sc")
                    packed = n <= 256
                    if not packed:
                        nc.tensor.matmul(sc[:, 0, :n], kh_t, qhs[:, h0, lo:],
                                         start=True, stop=True,
                                         skip_group_check=True)
                        nc.tensor.matmul(sc[:, 1, :n], kh_t, qhs[:, h1, lo:],
                                         start=True, stop=True,
                                         skip_group_check=True)
                    else:
                        # both heads in one matmul via strided moving operand
                        nc.tensor.matmul(sc[:, 0, :2 * n], kh_t,
                                         qhs[:, h0:h0 + 2, lo:],
                                         start=True, stop=True,
                                         skip_group_check=True)
                    probs = probsp.tile([128, 2, 512], BF16, tag="pr")
                    if not packed:
                        nc.scalar.activation(
                            probs[:, :, :n], sc[:, :, :n],
                            mybir.ActivationFunctionType.Exp, scale=SCALE)
                        pv = probs[:, :, :n]
                    else:
                        nc.scalar.activation(
                            probs[:, 0, :2 * n], sc[:, 0, :2 * n],
                            mybir.ActivationFunctionType.Exp, scale=SCALE)
                        pv = probs[:, 0, :2 * n].rearrange(
                            "p (g m) -> p g m", g=2)
                    if mode == "causal":
                        nc.vector.tensor_mul(
                            pv[:, :, 0:128], pv[:, :, 0:128],
                            m01b_s[:, kc, :].rearrange(
                                "p (g m) -> p g m", g=2))
                    elif mode == "mask":
                        for g in range(2):
                            nc.vector.tensor_mul(pv[:, g, :], pv[:, g, :],
                                                 m01s[:, kc, lo:])
                    if kc == 0:
                        nc.vector.tensor_copy(acc, pv)
                    else:
                        nc.vector.tensor_add(acc[:, :, lo:], acc[:, :, lo:],
                                             pv)
                    if prev is not None:
                        pkc, plo, pn, ppv = prev
                        for g in range(2):
                            nc.tensor.matmul(
                                ps_o[g][:, plo:],
                                vhs[:, pkc, hk * 128:(hk + 1) * 128],
                                ppv[:, g, :],
                                start=(pkc == 0), stop=(pkc == NKC - 1),
                                skip_group_check=True)
                    prev = (kc, lo, n, pv)
                    if kc == 2 and prev_acc is not None:
                        rowsums(P - 1, prev_acc)
                    if kc == 6 and P >= 3:
                        norm_pass(P - 3)    # overlapped DRAM roundtrip
                pkc, plo, pn, ppv = prev
                for g in range(2):
                    nc.tensor.matmul(
                        ps_o[g][:, plo:],
                        vhs[:, pkc, hk * 128:(hk + 1) * 128],
                        ppv[:, g, :],
                        start=(pkc == 0), stop=(pkc == NKC - 1),
                        skip_group_check=True)
                nc.vector.tensor_copy(outu(h0), ps_o[0])
                nc.vector.tensor_copy(outu(h1), ps_o[1])
                prev_acc = acc
                if P == 5:
                    # prefetch the first out-proj weight block
                    wo_t = stream2m.tile([128, H, 512], BF16, tag="s2m")
                    nc.sync.dma_start(out=wo_t, in_=wo[0].rearrange(
                        "p (h m) -> p h m", h=H))
                    wo_tiles[0] = wo_t
            rowsums(7, prev_acc)
            norm_pass(5)
            norm_pass(6)
            norm_pass(7)

            # ---------------- Phase D: out-projection ----------------
            for oc in range(4):
                if oc in wo_tiles:
                    wo_all = wo_tiles[oc]
                else:
                    wo_all = stream2m.tile([128, H, 512], BF16, tag="s2m")
                    nc.sync.dma_start(out=wo_all, in_=wo[oc].rearrange(
                        "p (h m) -> p h m", h=H))
                for qc in range(NQB):
                    ps_f = psA.tile([128, 512], F32, tag="mm")
                    for h in range(H):
                        lh = outu_a[:, h, qc * 128:(qc + 1) * 128] if h < 12 \
                            else outu_b[:, h - 12, qc * 128:(qc + 1) * 128]
                        nc.tensor.matmul(
                            ps_f, lh, wo_all[:, h, :],
                            start=(h == 0), stop=(h == H - 1))
                    fin = small.tile([128, 512], F32, tag="fin", bufs=2)
                    nc.vector.tensor_copy(fin, ps_f)
                    nc.sync.dma_start(
                        out=out[qc * 128:(qc + 1) * 128,
                                oc * 512:(oc + 1) * 512],
                        in_=fin)

    nc.compile()
    return nc


def _get_nc(mode: str):
    if mode not in _NC_CACHE:
        _NC_CACHE[mode] = _build(mode)
    return _NC_CACHE[mode]


def _core_rows(mode: str, r: int) -> np.ndarray:
    """Global (within-batch) q-row indices owned by quarter r, ascending."""
    if mode == "causal":
        blocks = sorted([r, 7 - r, 8 + r, 15 - r])
    else:
        blocks = [4 * r, 4 * r + 1, 4 * r + 2, 4 * r + 3]
    return np.concatenate([np.arange(b * 128, (b + 1) * 128) for b in blocks])


def kernel(q, k, v, mask, freqs, W_q, W_k, W_v, W_o):
    q = np.asarray(q, dtype=np.float32)
    k = np.asarray(k, dtype=np.float32)
    v = np.asarray(v, dtype=np.float32)
    mask = np.asarray(mask, dtype=np.float32)
    freqs = np.asarray(freqs, dtype=np.float32)
    W_q = np.asarray(W_q, dtype=np.float32)
    W_k = np.asarray(W_k, dtype=np.float32)
    W_v = np.asarray(W_v, dtype=np.float32)
    W_o = np.asarray(W_o, dtype=np.float32)

    # ---- mask mode detection ----
    nz = mask != 0
    if nz.all():
        mode = "none"
    else:
        tril = np.tril(np.ones((S, S), dtype=bool))
        mode = "causal" if all(np.array_equal(nz[b], tril) for b in range(B)) \
            else "mask"
    n_list = _n_list(mode)

    # ---- shared host precomputation ----
    c_full = np.cos(freqs)                      # [S, 64]
    s_full = np.sin(freqs)
    sgn = np.tile(np.array([-1.0, 1.0], np.float32), DH // 2)  # [-,+,-,+...]
    cosk_h = np.repeat(c_full, 2, axis=1).T.astype(BF)          # [128, S]
    sink_h = (np.repeat(s_full, 2, axis=1) * sgn).T.astype(BF)

    psw = np.zeros((128, 128), np.float32)
    idx = np.arange(128)
    psw[idx, idx ^ 1] = 1.0
    psw = psw.astype(BF)

    # weight layouts
    # wq[oc, p, i*128+m] = W_q[oc*128+m, i*128+p]
    wq_h = np.ascontiguousarray(
        W_q.reshape(H, 128, NIC, 128).transpose(0, 3, 2, 1)
        .reshape(H, 128, D)).astype(BF)
    # wk[hk, p, i*128+m] = W_k[hk*128+m, i*128+p]
    wk_h = np.ascontiguousarray(
        W_k.reshape(HKV, 128, NIC, 128).transpose(0, 3, 2, 1)
        .reshape(HKV, 128, D)).astype(BF)
    # wv[p, i*512+n] = W_v[n, i*128+p]
    wv_h = np.ascontiguousarray(
        W_v.reshape(DKV, NIC, 128).transpose(2, 1, 0).reshape(128, NIC * DKV)
    ).astype(BF)
    # wo[oc, p, h*512+m] = W_o[oc*512+m, h*128+p]
    wo_h = np.ascontiguousarray(
        W_o.reshape(4, 512, H, 128).transpose(0, 3, 2, 1).reshape(4, 128, -1)
    ).astype(BF)

    # k/v: each core only gets its own 512-token quarter (gathered on device)
    # kt[p, i*512+t] = k[b, tq*512+t, i*128+p] for quarter tq
    kt_b = []   # [B][4] quarters
    vt_b = []
    for b in range(B):
        kt_b.append([np.ascontiguousarray(
            k[b, tq * 512:(tq + 1) * 512].reshape(512, NIC, 128)
            .transpose(2, 1, 0).reshape(128, NIC * 512)).astype(BF)
            for tq in range(4)])
        # vt[j, p, i*128+t] = v[b, tq*512 + j*128+t, i*128+p]
        vt_b.append([np.ascontiguousarray(
            v[b, tq * 512:(tq + 1) * 512].reshape(4, 128, NIC, 128)
            .transpose(0, 3, 2, 1).reshape(4, 128, NIC * 128)).astype(BF)
            for tq in range(4)])

    in_maps = []
    rows_all = []
    for c in range(NCORES):
        b, r = divmod(c, 4)
        rows = _core_rows(mode, r)
        rows_all.append((b, rows))
        # qt[p, i*512+t] = q[b, rows[t], i*128+p]
        qsl = q[b][rows]                       # [512, D]
        qt_h = np.ascontiguousarray(
            qsl.reshape(RPC, NIC, 128).transpose(2, 1, 0).reshape(128, -1)
        ).astype(BF)
        cq = np.repeat(c_full[rows], 2, axis=1).T.astype(BF)      # [128, 512]
        sq = (np.repeat(s_full[rows], 2, axis=1) * sgn).T.astype(BF)
        im = {
            "wq": wq_h, "qt": qt_h, "kt": kt_b[b][r], "vt": vt_b[b][r],
            "wk": wk_h, "wv": wv_h, "wo": wo_h,
            "cosq": cq, "sinq": sq,
            "cosk": np.ascontiguousarray(cosk_h[:, r * 512:(r + 1) * 512]),
            "sink": np.ascontiguousarray(sink_h[:, r * 512:(r + 1) * 512]),
            "pswap": psw,
        }
        if mode == "causal":
            # m01b[p, kc*256 + g*128 + j] = mask[b, rows[lo_kc + j], kc*128+p]
            # (bottom 128-q stripe of the kept suffix, duplicated for pairs)
            msl = nz[b][rows]                  # [512, S] bool
            m01b_h = np.empty((128, NKC * 256), np.float32)
            for kc in range(NKC):
                lo = RPC - n_list[kc]
                pat = msl[lo:lo + 128, kc * 128:(kc + 1) * 128].T  # [128k,128q]
                m01b_h[:, kc * 256:kc * 256 + 128] = pat
                m01b_h[:, kc * 256 + 128:(kc + 1) * 256] = pat
            im["m01b"] = m01b_h.astype(BF)
        elif mode == "mask":
            # m01[p, kc*512+m] = (mask[b, rows[m], kc*128+p] != 0)
            msl = nz[b][rows]                  # [512, S] bool
            m01_h = np.ascontiguousarray(
                msl.T.reshape(NKC, 128, RPC).transpose(1, 0, 2)
                .reshape(128, -1)).astype(BF)
            im["m01"] = m01_h
        in_maps.append(im)

    nc = _get_nc(mode)
    kwargs = {}
    if TRACE:
        kwargs["trace"] = True
        if TRACE_CORES:
            kwargs["trace_cores"] = list(TRACE_CORES)
    results = run_bass_kernel_spmd(nc, in_maps, core_ids=list(range(NCORES)),
                                   **kwargs)
    global LAST_RESULTS
    LAST_RESULTS = results

    full = np.empty((B, S, D), np.float32)
    for c in range(NCORES):
        b, rows = rows_all[c]
        full[b, rows] = results.results[c]["out"]
    return full
